# revision 6
# baseline (speedup 1.0000x reference)
"""Trainium2 Bass kernel for the Laplace-kernel feature expansion.

Reference computation (per scalar x of the [16, 64, 64, 64] input):
    phi_i  = exp(-|x - p_i|)            for 15 design points p_i
    out_j  = sum_i chol_inv[i, j] phi_i
scattered so out[b, c*15 + j, h, w] comes from x[b, c, h, w].

Distribution: pure data parallel, 2 batches per core across 8 cores.

Per-core dataflow (all within one NeuronCore, no collectives):
  1. x is pre-split on host into bf16 (hi, lo) pairs, DMA'd in compactly.
  2. A TensorE "broadcast" matmul with a 0/1 block matrix replicates each
     x value onto 15 partitions (8 channel groups x 15 = 120 partitions),
     reconstructing fp32 x = hi + lo in PSUM; an extra ones-row in the
     moving operand makes the same matmul subtract the design point p_i
     (exact: the p_i are multiples of 0.25, exactly representable in bf16).
  3. VectorE computes |T| in one tensor_scalar op (abs_max with 0).
  4. ScalarE computes exp(-|t|) -> bf16.
  5. TensorE applies block-diag(chol_inv) -> PSUM (fp32).
  6. PSUM evicted to SBUF (split between ScalarE/VectorE), DMA to DRAM.
"""

import sys

if "/opt/trn_rl_repo" not in sys.path:
    sys.path.insert(0, "/opt/trn_rl_repo")

import numpy as np
import ml_dtypes

BF16 = ml_dtypes.bfloat16

B, C, H, W = 16, 64, 64, 64
P = H * W                # 4096 spatial positions
M_PTS = 15               # design points
G = 8                    # channel groups per tile
MROWS = G * M_PTS        # 120 partitions used
NCORES = 8
BPC = B // NCORES        # batches per core (2)
CBLK = C // G            # channel-block tiles per batch (8)
CHUNK = 1024             # elementwise free-dim chunk
NCHUNK = P // CHUNK      # 4
MMN = 512                # matmul moving free dim (PSUM bank limit)

# Fraction of PSUM->SBUF evictions done on VectorE (rest on ScalarE); the
# balance point from the engine cost model (DVE also does the abs pass).
DVE_EVICT_NUM = 25
TOTAL_CHUNKS = BPC * CBLK * NCHUNK  # 64

_CACHED = {}


def _build_nc():
    from concourse import bacc
    import concourse.mybir as mybir
    from concourse.tile import TileContext

    dt = mybir.dt
    Act = mybir.ActivationFunctionType
    Alu = mybir.AluOpType

    nc = bacc.Bacc(
        "TRN2", target_bir_lowering=False, debug=False, num_devices=NCORES
    )
    x_hl = nc.declare_dram_parameter(
        "x_hl", [BPC, CBLK, 2 * G + 1, P], dt.bfloat16, isOutput=False
    )
    w_bc = nc.declare_dram_parameter(
        "w_bc", [2 * G + 1, MROWS], dt.bfloat16, isOutput=False
    )
    r_blk = nc.declare_dram_parameter(
        "r_blk", [MROWS, MROWS], dt.bfloat16, isOutput=False
    )
    out = nc.declare_dram_parameter(
        "out", [BPC, C * M_PTS, P], dt.float32, isOutput=True
    )

    with TileContext(nc) as tc:
        with (
            tc.tile_pool(name="const", bufs=1) as cpool,
            tc.tile_pool(name="xin", bufs=3) as xpool,
            tc.tile_pool(name="absT", bufs=3) as apool,
            tc.tile_pool(name="phi", bufs=3) as ppool,
            tc.tile_pool(name="osb", bufs=2) as opool,
            tc.tile_pool(name="psT", bufs=2, space="PSUM") as psTp,
            tc.tile_pool(name="psO", bufs=2, space="PSUM") as psOp,
        ):
            w_t = cpool.tile([2 * G + 1, MROWS], dt.bfloat16)
            nc.sync.dma_start(out=w_t[:], in_=w_bc[:, :])
            r_t = cpool.tile([MROWS, MROWS], dt.bfloat16)
            nc.sync.dma_start(out=r_t[:], in_=r_blk[:, :])

            gc = 0
            for b in range(BPC):
                for cb in range(CBLK):
                    xt = xpool.tile([2 * G + 1, P], dt.bfloat16)
                    nc.sync.dma_start(out=xt[:], in_=x_hl[b, cb])
                    ot = opool.tile([MROWS, P], dt.float32)
                    for ch in range(NCHUNK):
                        tps = psTp.tile([MROWS, CHUNK], dt.float32)
                        for h in range(CHUNK // MMN):
                            nc.tensor.matmul(
                                tps[:, h * MMN : (h + 1) * MMN],
                                w_t[:],
                                xt[:, ch * CHUNK + h * MMN : ch * CHUNK + (h + 1) * MMN],
                                start=True,
                                stop=True,
                            )
                        at = apool.tile([MROWS, CHUNK], dt.float32)
                        # |T| via sign-bit clear on an int32 view (one DVE op)
                        nc.vector.tensor_scalar(
                            out=at[:].bitcast(dt.int32),
                            in0=tps[:].bitcast(dt.int32),
                            scalar1=0x7FFFFFFF,
                            scalar2=None,
                            op0=Alu.bitwise_and,
                        )
                        pt = ppool.tile([MROWS, CHUNK], dt.bfloat16)
                        nc.scalar.activation(pt[:], at[:], Act.Exp, scale=-1.0)
                        ops = psOp.tile([MROWS, CHUNK], dt.float32)
                        for h in range(CHUNK // MMN):
                            nc.tensor.matmul(
                                ops[:, h * MMN : (h + 1) * MMN],
                                r_t[:],
                                pt[:, h * MMN : (h + 1) * MMN],
                                start=True,
                                stop=True,
                            )
                        dst = ot[:, ch * CHUNK : (ch + 1) * CHUNK]
                        if (gc * DVE_EVICT_NUM) % TOTAL_CHUNKS < DVE_EVICT_NUM:
                            nc.vector.tensor_copy(out=dst, in_=ops[:])
                        else:
                            nc.scalar.activation(dst, ops[:], Act.Copy)
                        gc += 1
                    nc.sync.dma_start(
                        out=out[b, cb * MROWS : (cb + 1) * MROWS, :], in_=ot[:]
                    )
    nc.compile()
    return nc


def _host_prep(x, design_points, chol_inv):
    """Build the derived host-side arrays fed to the device."""
    xs = np.ascontiguousarray(np.asarray(x, dtype=np.float32)).reshape(B, C, P)
    x_hi = xs.astype(BF16)
    x_lo = (xs - x_hi.astype(np.float32)).astype(BF16)
    # [B, CBLK, 2G+1, P]: hi/lo interleaved over the 8 channels of a block,
    # plus a ones-row so the broadcast matmul can subtract the design point.
    x_hl = np.empty((B, CBLK, 2 * G + 1, P), dtype=BF16)
    x_hl[:, :, 0:2 * G:2, :] = x_hi.reshape(B, CBLK, G, P)
    x_hl[:, :, 1:2 * G:2, :] = x_lo.reshape(B, CBLK, G, P)
    x_hl[:, :, 2 * G, :] = BF16(1.0)

    pts = np.asarray(design_points, dtype=np.float32)
    w_bc = np.zeros((2 * G + 1, MROWS), dtype=np.float32)
    for g in range(G):
        w_bc[2 * g, 15 * g : 15 * g + 15] = 1.0
        w_bc[2 * g + 1, 15 * g : 15 * g + 15] = 1.0
        w_bc[2 * G, 15 * g : 15 * g + 15] = -pts
    w_bc = w_bc.astype(BF16)

    chol = np.asarray(chol_inv, dtype=np.float32)
    r_blk = np.zeros((MROWS, MROWS), dtype=np.float32)
    for g in range(G):
        r_blk[15 * g : 15 * g + 15, 15 * g : 15 * g + 15] = chol
    r_blk = r_blk.astype(BF16)

    return x_hl, w_bc, r_blk


LAST_RESULT = None


def kernel(x, design_points, chol_inv):
    global LAST_RESULT
    from concourse.bass_utils import run_bass_kernel_spmd

    if "nc" not in _CACHED:
        _CACHED["nc"] = _build_nc()
    nc = _CACHED["nc"]

    x_hl, w_bc, r_blk = _host_prep(x, design_points, chol_inv)

    in_maps = []
    for core in range(NCORES):
        in_maps.append(
            {
                "x_hl": np.ascontiguousarray(x_hl[core * BPC : (core + 1) * BPC]),
                "w_bc": w_bc,
                "r_blk": r_blk,
            }
        )

    res = run_bass_kernel_spmd(nc, in_maps, core_ids=list(range(NCORES)))
    LAST_RESULT = res

    full = np.empty((B, C * M_PTS, P), dtype=np.float32)
    for core in range(NCORES):
        full[core * BPC : (core + 1) * BPC] = res.results[core]["out"]
    return full.reshape(B, C * M_PTS, H, W)
